# revision 4
# baseline (speedup 1.0000x reference)
"""Fused multi-head self-attention (concat-head, scale=sqrt(d_model)) on 8 trn2 cores.

Sharding: batch(4) x query-half(2) -> 8 cores. Each core:
  - inputs: xT of its batch [F=512, T=2048] (pre-transposed on host),
    xqT = query-half columns [512, 1024], Wq/Wk/Wv [512, 512].
  - computes qT = Wq^T @ xqT, kT = Wk^T @ xT, v = xT^T @ Wv
  - scoresT[s, tq] = kT.T @ qT (contract p), expT = exp(scoresT / sqrt(512))
    (no max-subtraction needed: scores are O(1))
  - out[tq, p] = expT.T @ v, normalized by row-sums obtained from a
    ones-vector matmul; normalization folded into the PSUM->SBUF copy.
All matmul operands are bf16 (fp32 accumulate); softmax/normalize in fp32.
"""

import os
from contextlib import ExitStack

import numpy as np
import ml_dtypes

import concourse.bass as bass
import concourse.tile as tile
import concourse.mybir as mybir
from concourse import bacc
from concourse.bass_utils import run_bass_kernel_spmd

B, T, F, P = 4, 2048, 512, 512
NCORES = 8
QSPLIT = NCORES // B          # query-dim split per batch
TQ = T // QSPLIT              # 1024 query rows per core
SCALE = 1.0 / float(np.sqrt(512.0))

FT = F // 128    # 4 f-tiles (contraction of projections)
PT = P // 128    # 4 p-tiles (contraction of scores)
ST = T // 128    # 16 s-tiles (keys)
NCH = TQ // 512  # 2 query chunks of 512
F32 = mybir.dt.float32

# matmul dtype: "bf16" (1 cyc/row) | "fp32" (4 cyc/row, exact)
KDT = os.environ.get("KDT", "bf16")


def _mm_dtypes():
    if KDT == "bf16":
        return mybir.dt.bfloat16, np.dtype(ml_dtypes.bfloat16)
    elif KDT == "fp32":
        return mybir.dt.float32, np.dtype(np.float32)
    else:
        raise ValueError(KDT)


def _attn_body(ctx, tc, xkvt, xqt, wq, wk, wv, out):
    nc = tc.nc
    DT, _ = _mm_dtypes()
    Exp = mybir.ActivationFunctionType.Exp
    Copy = mybir.ActivationFunctionType.Copy

    consts = ctx.enter_context(tc.tile_pool(name="consts", bufs=1))
    persist = ctx.enter_context(tc.tile_pool(name="persist", bufs=1))
    exp_pool = ctx.enter_context(tc.tile_pool(name="expp", bufs=2))
    out_pool = ctx.enter_context(tc.tile_pool(name="outsb", bufs=3))
    small = ctx.enter_context(tc.tile_pool(name="small", bufs=2))
    ps_mm = ctx.enter_context(tc.tile_pool(name="psmm", bufs=4, space="PSUM"))
    ps_sum = ctx.enter_context(tc.tile_pool(name="pssum", bufs=2, space="PSUM"))
    ps_sumt = ctx.enter_context(tc.tile_pool(name="pssumt", bufs=2, space="PSUM"))

    # ---- load weights + inputs ----
    wq_sb = [consts.tile([128, P], DT, tag=f"wq{i}", name=f"wq{i}") for i in range(FT)]
    wk_sb = [consts.tile([128, P], DT, tag=f"wk{i}", name=f"wk{i}") for i in range(FT)]
    wv_sb = [consts.tile([128, P], DT, tag=f"wv{i}", name=f"wv{i}") for i in range(FT)]
    xq_sb = [consts.tile([128, TQ], DT, tag=f"xq{i}", name=f"xq{i}") for i in range(FT)]
    xkv_sb = [
        consts.tile([128, T], DT, tag=f"xkv{i}", name=f"xkv{i}") for i in range(FT)
    ]
    for i in range(FT):
        nc.sync.dma_start(out=wq_sb[i], in_=wq[i * 128 : (i + 1) * 128, :])
        nc.sync.dma_start(out=xq_sb[i], in_=xqt[i * 128 : (i + 1) * 128, :])
    for i in range(FT):
        nc.sync.dma_start(out=wk_sb[i], in_=wk[i * 128 : (i + 1) * 128, :])
        nc.sync.dma_start(out=xkv_sb[i], in_=xkvt[i * 128 : (i + 1) * 128, :])
    for i in range(FT):
        nc.sync.dma_start(out=wv_sb[i], in_=wv[i * 128 : (i + 1) * 128, :])

    ones_sb = consts.tile([128, 1], DT, tag="ones", name="ones")
    nc.vector.memset(ones_sb, 1.0)
    onef_sb = consts.tile([1, 1], F32, tag="onef", name="onef")
    nc.vector.memset(onef_sb, 1.0)

    # ---- projections ----
    qt_sb = [
        persist.tile([128, TQ], DT, tag=f"qt{m}", name=f"qt{m}") for m in range(PT)
    ]
    for m in range(PT):
        for c in range(TQ // 512):
            ps = ps_mm.tile([128, 512], F32, tag="mm", name="ps_q")
            for kf in range(FT):
                nc.tensor.matmul(
                    ps,
                    wq_sb[kf][:, m * 128 : (m + 1) * 128],
                    xq_sb[kf][:, c * 512 : (c + 1) * 512],
                    start=kf == 0,
                    stop=kf == FT - 1,
                )
            nc.any.tensor_copy(out=qt_sb[m][:, c * 512 : (c + 1) * 512], in_=ps)

    kt_sb = [
        persist.tile([128, T], DT, tag=f"kt{m}", name=f"kt{m}") for m in range(PT)
    ]
    for m in range(PT):
        for c in range(T // 512):
            ps = ps_mm.tile([128, 512], F32, tag="mm", name="ps_k")
            for kf in range(FT):
                nc.tensor.matmul(
                    ps,
                    wk_sb[kf][:, m * 128 : (m + 1) * 128],
                    xkv_sb[kf][:, c * 512 : (c + 1) * 512],
                    start=kf == 0,
                    stop=kf == FT - 1,
                )
            nc.any.tensor_copy(out=kt_sb[m][:, c * 512 : (c + 1) * 512], in_=ps)

    v_sb = [persist.tile([128, P], DT, tag=f"v{s}", name=f"v{s}") for s in range(ST)]
    for s in range(ST):
        ps = ps_mm.tile([128, 512], F32, tag="mm", name="ps_v")
        for kf in range(FT):
            nc.tensor.matmul(
                ps,
                xkv_sb[kf][:, s * 128 : (s + 1) * 128],
                wv_sb[kf],
                start=kf == 0,
                stop=kf == FT - 1,
            )
        nc.any.tensor_copy(out=v_sb[s], in_=ps)

    # ---- attention, per query chunk of 512 ----
    for c in range(NCH):
        qs = slice(c * 512, (c + 1) * 512)
        exp_c = [
            exp_pool.tile([128, 512], DT, tag=f"exp{s}", name=f"exp{s}")
            for s in range(ST)
        ]
        sums_ps = ps_sum.tile([1, 512], F32, tag="sums", name="sums_ps")
        for s in range(ST):
            ps = ps_mm.tile([128, 512], F32, tag="mm", name="ps_sc")
            for pm in range(PT):
                nc.tensor.matmul(
                    ps,
                    kt_sb[pm][:, s * 128 : (s + 1) * 128],
                    qt_sb[pm][:, qs],
                    start=pm == 0,
                    stop=pm == PT - 1,
                )
            nc.scalar.activation(out=exp_c[s], in_=ps, func=Exp, scale=SCALE)
            # row-sums of exp over s (partition dim) via ones-matmul
            nc.tensor.matmul(
                sums_ps,
                ones_sb,
                exp_c[s],
                start=s == 0,
                stop=s == ST - 1,
                skip_group_check=True,
            )

        # transpose sums [1, 512] -> [128, 4] via 4 tiny matmuls, reciprocal
        sums_sb = small.tile([1, 512], F32, tag="sums_sb", name="sums_sb")
        nc.any.tensor_copy(out=sums_sb, in_=sums_ps)
        sumt_ps = ps_sumt.tile([128, 4], F32, tag="sumt", name="sumt_ps")
        for t4 in range(4):
            nc.tensor.matmul(
                sumt_ps[:, t4 : t4 + 1],
                sums_sb[0:1, t4 * 128 : (t4 + 1) * 128],
                onef_sb,
                start=True,
                stop=True,
                skip_group_check=True,
            )
        recip_sb = small.tile([128, 4], F32, tag="recip", name="recip_sb")
        nc.vector.reciprocal(recip_sb, sumt_ps)

        for t4 in range(4):
            tt = c * 4 + t4
            po = ps_mm.tile([128, 512], F32, tag="mm", name="ps_o")
            for s in range(ST):
                nc.tensor.matmul(
                    po,
                    exp_c[s][:, t4 * 128 : (t4 + 1) * 128],
                    v_sb[s],
                    start=s == 0,
                    stop=s == ST - 1,
                )
            osb = out_pool.tile([128, 512], F32, tag="osb", name="osb")
            nc.scalar.activation(
                out=osb, in_=po, func=Copy, scale=recip_sb[:, t4 : t4 + 1]
            )
            nc.sync.dma_start(out=out[tt * 128 : (tt + 1) * 128, :], in_=osb)


_CACHE = {}


def _get_compiled():
    key = KDT
    if key in _CACHE:
        return _CACHE[key]
    DT, _ = _mm_dtypes()
    nc = bacc.Bacc(
        "TRN2",
        target_bir_lowering=False,
        debug=False,
        enable_asserts=False,
        num_devices=NCORES,
    )
    xkvt = nc.dram_tensor("xkvt", [F, T], DT, kind="ExternalInput").ap()
    xqt = nc.dram_tensor("xqt", [F, TQ], DT, kind="ExternalInput").ap()
    wq = nc.dram_tensor("wq", [F, P], DT, kind="ExternalInput").ap()
    wk = nc.dram_tensor("wk", [F, P], DT, kind="ExternalInput").ap()
    wv = nc.dram_tensor("wv", [F, P], DT, kind="ExternalInput").ap()
    out = nc.dram_tensor("out", [TQ, P], F32, kind="ExternalOutput").ap()
    with tile.TileContext(nc) as tc, ExitStack() as ctx:
        _attn_body(ctx, tc, xkvt, xqt, wq, wk, wv, out)
    nc.compile()
    _CACHE[key] = nc
    return nc


def kernel(x, Wq, Wk, Wv, _trace=False):
    _, np_dt = _mm_dtypes()
    nc = _get_compiled()
    wq_c = np.ascontiguousarray(Wq.astype(np_dt))
    wk_c = np.ascontiguousarray(Wk.astype(np_dt))
    wv_c = np.ascontiguousarray(Wv.astype(np_dt))
    xT = [np.ascontiguousarray(x[b].T.astype(np_dt)) for b in range(B)]
    in_maps = []
    for core in range(NCORES):
        b, h = divmod(core, QSPLIT)
        in_maps.append(
            {
                "xkvt": xT[b],
                "xqt": np.ascontiguousarray(xT[b][:, h * TQ : (h + 1) * TQ]),
                "wq": wq_c,
                "wk": wk_c,
                "wv": wv_c,
            }
        )
    res = run_bass_kernel_spmd(
        nc, in_maps, core_ids=list(range(NCORES)), trace=_trace
    )
    out = np.empty((B, T, P), np.float32)
    for core in range(NCORES):
        b, h = divmod(core, QSPLIT)
        out[b, h * TQ : (h + 1) * TQ, :] = res.results[core]["out"]
    if _trace:
        return out, res
    return out


# revision 6
# speedup vs baseline: 1.0393x; 1.0393x over previous
"""Fused multi-head self-attention (concat-head, scale=sqrt(d_model)) on 8 trn2 cores.

Sharding: batch(4) x key-half(2) -> 8 cores. Each core:
  - inputs: xqT = x[b].T [F=512, T=2048] (pre-transposed on host),
    xkvT = its key-half columns [512, 1024], Wq/Wk/Wv [512, 512].
  - computes qT = Wq^T @ xqT (all queries), kT = Wk^T @ xkvT, v = xkvT^T @ Wv
    (its 1024 keys only)
  - scoresT[s, tq] = kT.T @ qT (contract p), expT = exp(scoresT / sqrt(512))
    (no max-subtraction needed: scores are O(1))
  - partial out[tq, p] = expT.T @ v and partial row-sums (ones-vector matmul),
    both returned unnormalized; host combines the two key-halves:
    out = (o0 + o1) / (s0 + s1).
All matmul operands are bf16 (fp32 accumulate); exp/sums in fp32.
"""

import os
from contextlib import ExitStack

import numpy as np
import ml_dtypes

import concourse.bass as bass
import concourse.tile as tile
import concourse.mybir as mybir
from concourse import bacc
from concourse.bass_utils import run_bass_kernel_spmd

B, T, F, P = 4, 2048, 512, 512
NCORES = 8
KSPLIT = NCORES // B          # key-dim split per batch
TKV = T // KSPLIT             # 1024 keys per core
SCALE = 1.0 / float(np.sqrt(512.0))

FT = F // 128     # 4 f-tiles (contraction of projections)
PT = P // 128     # 4 p-tiles (contraction of scores)
ST = TKV // 128   # 8 s-tiles (keys per core)
NCH = T // 512    # 4 query chunks of 512
F32 = mybir.dt.float32

# matmul dtype: "bf16" (1 cyc/row) | "fp32" (4 cyc/row, exact)
KDT = os.environ.get("KDT", "bf16")


def _mm_dtypes():
    if KDT == "bf16":
        return mybir.dt.bfloat16, np.dtype(ml_dtypes.bfloat16)
    elif KDT == "fp32":
        return mybir.dt.float32, np.dtype(np.float32)
    else:
        raise ValueError(KDT)


def _attn_body(ctx, tc, xqt, xkvt, wq, wk, wv, out, sums):
    nc = tc.nc
    DT, _ = _mm_dtypes()
    Exp = mybir.ActivationFunctionType.Exp

    consts = ctx.enter_context(tc.tile_pool(name="consts", bufs=1))
    persist = ctx.enter_context(tc.tile_pool(name="persist", bufs=1))
    exp_pool = ctx.enter_context(tc.tile_pool(name="expp", bufs=2))
    out_pool = ctx.enter_context(tc.tile_pool(name="outsb", bufs=3))
    small = ctx.enter_context(tc.tile_pool(name="small", bufs=2))
    ps_mm = ctx.enter_context(tc.tile_pool(name="psmm", bufs=6, space="PSUM"))
    ps_sum = ctx.enter_context(tc.tile_pool(name="pssum", bufs=2, space="PSUM"))

    # ---- load weights + inputs (wq/xq first so qT matmuls start early) ----
    wq_sb = [consts.tile([128, P], DT, tag=f"wq{i}", name=f"wq{i}") for i in range(FT)]
    wk_sb = [consts.tile([128, P], DT, tag=f"wk{i}", name=f"wk{i}") for i in range(FT)]
    wv_sb = [consts.tile([128, P], DT, tag=f"wv{i}", name=f"wv{i}") for i in range(FT)]
    xq_sb = [consts.tile([128, T], DT, tag=f"xq{i}", name=f"xq{i}") for i in range(FT)]
    xkv_sb = [
        consts.tile([128, TKV], DT, tag=f"xkv{i}", name=f"xkv{i}") for i in range(FT)
    ]
    dma_engines = [nc.sync, nc.scalar, nc.sync, nc.scalar]
    for i in range(FT):
        nc.sync.dma_start(out=wq_sb[i], in_=wq[i * 128 : (i + 1) * 128, :])
        dma_engines[i].dma_start(out=xq_sb[i], in_=xqt[i * 128 : (i + 1) * 128, :])
    for i in range(FT):
        nc.sync.dma_start(out=wk_sb[i], in_=wk[i * 128 : (i + 1) * 128, :])
        dma_engines[i].dma_start(out=xkv_sb[i], in_=xkvt[i * 128 : (i + 1) * 128, :])
    for i in range(FT):
        nc.sync.dma_start(out=wv_sb[i], in_=wv[i * 128 : (i + 1) * 128, :])

    ones_sb = consts.tile([128, 1], DT, tag="ones", name="ones")
    nc.vector.memset(ones_sb, 1.0)

    # ---- projections ----
    qt_sb = [persist.tile([128, T], DT, tag=f"qt{m}", name=f"qt{m}") for m in range(PT)]
    for m in range(PT):
        for c in range(T // 512):
            ps = ps_mm.tile([128, 512], F32, tag="mm", name="ps_q")
            for kf in range(FT):
                nc.tensor.matmul(
                    ps,
                    wq_sb[kf][:, m * 128 : (m + 1) * 128],
                    xq_sb[kf][:, c * 512 : (c + 1) * 512],
                    start=kf == 0,
                    stop=kf == FT - 1,
                )
            nc.vector.tensor_copy(out=qt_sb[m][:, c * 512 : (c + 1) * 512], in_=ps)

    kt_sb = [
        persist.tile([128, TKV], DT, tag=f"kt{m}", name=f"kt{m}") for m in range(PT)
    ]
    for m in range(PT):
        for c in range(TKV // 512):
            ps = ps_mm.tile([128, 512], F32, tag="mm", name="ps_k")
            for kf in range(FT):
                nc.tensor.matmul(
                    ps,
                    wk_sb[kf][:, m * 128 : (m + 1) * 128],
                    xkv_sb[kf][:, c * 512 : (c + 1) * 512],
                    start=kf == 0,
                    stop=kf == FT - 1,
                )
            nc.vector.tensor_copy(out=kt_sb[m][:, c * 512 : (c + 1) * 512], in_=ps)

    v_sb = [persist.tile([128, P], DT, tag=f"v{s}", name=f"v{s}") for s in range(ST)]
    for s in range(ST):
        ps = ps_mm.tile([128, 512], F32, tag="mm", name="ps_v")
        for kf in range(FT):
            nc.tensor.matmul(
                ps,
                xkv_sb[kf][:, s * 128 : (s + 1) * 128],
                wv_sb[kf],
                start=kf == 0,
                stop=kf == FT - 1,
            )
        nc.vector.tensor_copy(out=v_sb[s], in_=ps)

    # ---- attention, per query chunk of 512 ----
    for c in range(NCH):
        qs = slice(c * 512, (c + 1) * 512)
        exp_c = [
            exp_pool.tile([128, 512], DT, tag=f"exp{s}", name=f"exp{s}")
            for s in range(ST)
        ]
        sums_ps = ps_sum.tile([1, 512], F32, tag="sums", name="sums_ps")
        for s in range(ST):
            ps = ps_mm.tile([128, 512], F32, tag="mm", name="ps_sc")
            for pm in range(PT):
                nc.tensor.matmul(
                    ps,
                    kt_sb[pm][:, s * 128 : (s + 1) * 128],
                    qt_sb[pm][:, qs],
                    start=pm == 0,
                    stop=pm == PT - 1,
                )
            nc.scalar.activation(out=exp_c[s], in_=ps, func=Exp, scale=SCALE)
            # partial row-sums of exp over s (partition dim) via ones-matmul
            nc.tensor.matmul(
                sums_ps,
                ones_sb,
                exp_c[s],
                start=s == 0,
                stop=s == ST - 1,
                skip_group_check=True,
            )

        sums_sb = small.tile([1, 512], F32, tag="sums_sb", name="sums_sb")
        nc.vector.tensor_copy(out=sums_sb, in_=sums_ps)
        nc.sync.dma_start(out=sums[0:1, qs], in_=sums_sb)

        for t4 in range(4):
            tt = c * 4 + t4
            po = ps_mm.tile([128, 512], F32, tag="mm", name="ps_o")
            for s in range(ST):
                nc.tensor.matmul(
                    po,
                    exp_c[s][:, t4 * 128 : (t4 + 1) * 128],
                    v_sb[s],
                    start=s == 0,
                    stop=s == ST - 1,
                )
            osb = out_pool.tile([128, 512], F32, tag="osb", name="osb")
            nc.vector.tensor_copy(out=osb, in_=po)
            nc.sync.dma_start(out=out[tt * 128 : (tt + 1) * 128, :], in_=osb)


_CACHE = {}


def _get_compiled():
    key = KDT
    if key in _CACHE:
        return _CACHE[key]
    DT, _ = _mm_dtypes()
    nc = bacc.Bacc(
        "TRN2",
        target_bir_lowering=False,
        debug=False,
        enable_asserts=False,
        num_devices=NCORES,
    )
    xqt = nc.dram_tensor("xqt", [F, T], DT, kind="ExternalInput").ap()
    xkvt = nc.dram_tensor("xkvt", [F, TKV], DT, kind="ExternalInput").ap()
    wq = nc.dram_tensor("wq", [F, P], DT, kind="ExternalInput").ap()
    wk = nc.dram_tensor("wk", [F, P], DT, kind="ExternalInput").ap()
    wv = nc.dram_tensor("wv", [F, P], DT, kind="ExternalInput").ap()
    out = nc.dram_tensor("out", [T, P], F32, kind="ExternalOutput").ap()
    sums = nc.dram_tensor("sums", [1, T], F32, kind="ExternalOutput").ap()
    with tile.TileContext(nc) as tc, ExitStack() as ctx:
        _attn_body(ctx, tc, xqt, xkvt, wq, wk, wv, out, sums)
    nc.compile()
    _CACHE[key] = nc
    return nc


def kernel(x, Wq, Wk, Wv, _trace=False):
    _, np_dt = _mm_dtypes()
    nc = _get_compiled()
    wq_c = np.ascontiguousarray(Wq.astype(np_dt))
    wk_c = np.ascontiguousarray(Wk.astype(np_dt))
    wv_c = np.ascontiguousarray(Wv.astype(np_dt))
    xT = [np.ascontiguousarray(x[b].T.astype(np_dt)) for b in range(B)]
    in_maps = []
    for core in range(NCORES):
        b, h = divmod(core, KSPLIT)
        in_maps.append(
            {
                "xqt": xT[b],
                "xkvt": np.ascontiguousarray(xT[b][:, h * TKV : (h + 1) * TKV]),
                "wq": wq_c,
                "wk": wk_c,
                "wv": wv_c,
            }
        )
    res = run_bass_kernel_spmd(
        nc, in_maps, core_ids=list(range(NCORES)), trace=_trace
    )
    out = np.empty((B, T, P), np.float32)
    for b in range(B):
        r0 = res.results[b * KSPLIT]
        r1 = res.results[b * KSPLIT + 1]
        o = r0["out"] + r1["out"]
        s = r0["sums"][0] + r1["sums"][0]
        out[b] = o / s[:, None]
    if _trace:
        return out, res
    return out


# revision 8
# speedup vs baseline: 1.0591x; 1.0191x over previous
"""Fused multi-head self-attention (concat-head, scale=sqrt(d_model)) on 8 trn2 cores.

Sharding: batch(4) x key-half(2) -> 8 cores. Each core:
  - inputs: xqT = x[b].T [F=512, T=2048] (pre-transposed on host),
    xkvT = its key-half columns [512, 1024], Wq/Wk/Wv [512, 512].
  - computes qT = Wq^T @ xqT (all queries), kT = Wk^T @ xkvT, v = xkvT^T @ Wv
    (its 1024 keys only)
  - scoresT[s, tq] = kT.T @ qT (contract p), expT = exp(scoresT / sqrt(512))
    (no max-subtraction needed: scores are O(1))
  - partial out[tq, p] = expT.T @ v and partial row-sums (ones-vector matmul),
    both returned unnormalized; host combines the two key-halves:
    out = (o0 + o1) / (s0 + s1).
All matmul operands are bf16 (fp32 accumulate); exp/sums in fp32.
"""

import os
from contextlib import ExitStack

import numpy as np
import ml_dtypes

import concourse.bass as bass
import concourse.tile as tile
import concourse.mybir as mybir
from concourse import bacc
from concourse.bass_utils import run_bass_kernel_spmd

B, T, F, P = 4, 2048, 512, 512
NCORES = 8
KSPLIT = NCORES // B          # key-dim split per batch
TKV = T // KSPLIT             # 1024 keys per core
SCALE = 1.0 / float(np.sqrt(512.0))

FT = F // 128     # 4 f-tiles (contraction of projections)
PT = P // 128     # 4 p-tiles (contraction of scores)
ST = TKV // 128   # 8 s-tiles (keys per core)
NCH = T // 512    # 4 query chunks of 512
F32 = mybir.dt.float32

# matmul dtype: "bf16" (1 cyc/row) | "fp32" (4 cyc/row, exact)
KDT = os.environ.get("KDT", "bf16")


def _mm_dtypes():
    if KDT == "bf16":
        return mybir.dt.bfloat16, np.dtype(ml_dtypes.bfloat16)
    elif KDT == "fp32":
        return mybir.dt.float32, np.dtype(np.float32)
    else:
        raise ValueError(KDT)


def _attn_body(ctx, tc, xqt, xkvt, wq, wk, wv, out, sums):
    nc = tc.nc
    DT, _ = _mm_dtypes()
    Exp = mybir.ActivationFunctionType.Exp

    consts = ctx.enter_context(tc.tile_pool(name="consts", bufs=1))
    persist = ctx.enter_context(tc.tile_pool(name="persist", bufs=1))
    exp_pool = ctx.enter_context(tc.tile_pool(name="expp", bufs=2))
    out_pool = ctx.enter_context(tc.tile_pool(name="outsb", bufs=3))
    small = ctx.enter_context(tc.tile_pool(name="small", bufs=2))
    ps_sc = ctx.enter_context(tc.tile_pool(name="pssc", bufs=2, space="PSUM"))
    ps_out = ctx.enter_context(tc.tile_pool(name="psout", bufs=4, space="PSUM"))
    ps_sum = ctx.enter_context(tc.tile_pool(name="pssum", bufs=2, space="PSUM"))

    # ---- PE warmup: junk matmuls with no DMA deps, overlap the HAM ramp
    # and the initial input DMAs ----
    junk = consts.tile([128, 512], DT, tag="junk", name="junk")
    nc.vector.memset(junk, 0.0)
    for w in range(28):
        wu = ps_sc.tile([128, 128], F32, tag="sc", name="wu")
        nc.tensor.matmul(wu, junk[:, 0:128], junk[:, 0:128], start=True, stop=True)

    # ---- load weights + inputs (wq/xq chunk 0 first so qT starts early) ----
    wq_sb = [consts.tile([128, P], DT, tag=f"wq{i}", name=f"wq{i}") for i in range(FT)]
    wk_sb = [consts.tile([128, P], DT, tag=f"wk{i}", name=f"wk{i}") for i in range(FT)]
    wv_sb = [consts.tile([128, P], DT, tag=f"wv{i}", name=f"wv{i}") for i in range(FT)]
    xq_sb = [consts.tile([128, T], DT, tag=f"xq{i}", name=f"xq{i}") for i in range(FT)]
    xkv_sb = [
        consts.tile([128, TKV], DT, tag=f"xkv{i}", name=f"xkv{i}") for i in range(FT)
    ]
    qdma = [nc.sync, nc.scalar]

    def dma_in(idx, out_ap, in_ap):
        qdma[idx % 2].dma_start(out=out_ap, in_=in_ap)

    di = 0
    for i in range(FT):
        dma_in(di, wq_sb[i], wq[i * 128 : (i + 1) * 128, :])
        dma_in(di + 1, xq_sb[i][:, 0:512], xqt[i * 128 : (i + 1) * 128, 0:512])
        di += 2
    for c in range(1, T // 512):
        for i in range(FT):
            dma_in(
                di,
                xq_sb[i][:, c * 512 : (c + 1) * 512],
                xqt[i * 128 : (i + 1) * 128, c * 512 : (c + 1) * 512],
            )
            di += 1
    for i in range(FT):
        dma_in(di, wk_sb[i], wk[i * 128 : (i + 1) * 128, :])
        di += 1
    for c in range(TKV // 512):
        for i in range(FT):
            dma_in(
                di,
                xkv_sb[i][:, c * 512 : (c + 1) * 512],
                xkvt[i * 128 : (i + 1) * 128, c * 512 : (c + 1) * 512],
            )
            di += 1
    for i in range(FT):
        dma_in(di, wv_sb[i], wv[i * 128 : (i + 1) * 128, :])
        di += 1

    ones_sb = consts.tile([128, 1], DT, tag="ones", name="ones")
    nc.vector.memset(ones_sb, 1.0)

    # ---- projections (chunk-outer so the first chunk starts ASAP) ----
    qt_sb = [persist.tile([128, T], DT, tag=f"qt{m}", name=f"qt{m}") for m in range(PT)]
    for c in range(T // 512):
        for m in range(PT):
            ps = ps_sc.tile([128, 512], F32, tag="sc", name="ps_q")
            for kf in range(FT):
                nc.tensor.matmul(
                    ps,
                    wq_sb[kf][:, m * 128 : (m + 1) * 128],
                    xq_sb[kf][:, c * 512 : (c + 1) * 512],
                    start=kf == 0,
                    stop=kf == FT - 1,
                )
            nc.vector.tensor_copy(out=qt_sb[m][:, c * 512 : (c + 1) * 512], in_=ps)

    kt_sb = [
        persist.tile([128, TKV], DT, tag=f"kt{m}", name=f"kt{m}") for m in range(PT)
    ]
    for c in range(TKV // 512):
        for m in range(PT):
            ps = ps_sc.tile([128, 512], F32, tag="sc", name="ps_k")
            for kf in range(FT):
                nc.tensor.matmul(
                    ps,
                    wk_sb[kf][:, m * 128 : (m + 1) * 128],
                    xkv_sb[kf][:, c * 512 : (c + 1) * 512],
                    start=kf == 0,
                    stop=kf == FT - 1,
                )
            nc.vector.tensor_copy(out=kt_sb[m][:, c * 512 : (c + 1) * 512], in_=ps)

    v_sb = [persist.tile([128, P], DT, tag=f"v{s}", name=f"v{s}") for s in range(ST)]
    for s in range(ST):
        ps = ps_sc.tile([128, 512], F32, tag="sc", name="ps_v")
        for kf in range(FT):
            nc.tensor.matmul(
                ps,
                xkv_sb[kf][:, s * 128 : (s + 1) * 128],
                wv_sb[kf],
                start=kf == 0,
                stop=kf == FT - 1,
            )
        nc.vector.tensor_copy(out=v_sb[s], in_=ps)

    # ---- attention, per query chunk of 512; out-accumulation s-outer,
    # pipelined one s-step behind scores so PE never waits on ACT exp ----
    for c in range(NCH):
        qs = slice(c * 512, (c + 1) * 512)
        exp_c = [
            exp_pool.tile([128, 512], DT, tag=f"exp{s % 3}", name=f"exp{s % 3}")
            for s in range(ST)
        ]
        sums_ps = ps_sum.tile([1, 512], F32, tag="sums", name="sums_ps")
        po = [
            ps_out.tile([128, 512], F32, tag=f"out{t4}", name=f"po{t4}", bufs=1)
            for t4 in range(4)
        ]

        def scores_step(s):
            ps = ps_sc.tile([128, 512], F32, tag="sc", name="ps_sc")
            for pm in range(PT):
                nc.tensor.matmul(
                    ps,
                    kt_sb[pm][:, s * 128 : (s + 1) * 128],
                    qt_sb[pm][:, qs],
                    start=pm == 0,
                    stop=pm == PT - 1,
                )
            nc.scalar.activation(out=exp_c[s], in_=ps, func=Exp, scale=SCALE)

        def out_step(s):
            for t4 in range(4):
                nc.tensor.matmul(
                    po[t4],
                    exp_c[s][:, t4 * 128 : (t4 + 1) * 128],
                    v_sb[s],
                    start=s == 0,
                    stop=s == ST - 1,
                    skip_group_check=True,
                )
            # partial row-sums of exp over s (partition dim) via ones-matmul
            nc.tensor.matmul(
                sums_ps,
                ones_sb,
                exp_c[s],
                start=s == 0,
                stop=s == ST - 1,
                skip_group_check=True,
            )

        scores_step(0)
        for s in range(1, ST):
            scores_step(s)
            out_step(s - 1)
        out_step(ST - 1)

        sums_sb = small.tile([1, 512], F32, tag="sums_sb", name="sums_sb")
        nc.vector.tensor_copy(out=sums_sb, in_=sums_ps)
        nc.sync.dma_start(out=sums[0:1, qs], in_=sums_sb)

        for t4 in range(4):
            tt = c * 4 + t4
            osb = out_pool.tile([128, 512], F32, tag="osb", name="osb")
            nc.vector.tensor_copy(out=osb, in_=po[t4])
            nc.sync.dma_start(out=out[tt * 128 : (tt + 1) * 128, :], in_=osb)


_CACHE = {}


def _get_compiled():
    key = KDT
    if key in _CACHE:
        return _CACHE[key]
    DT, _ = _mm_dtypes()
    nc = bacc.Bacc(
        "TRN2",
        target_bir_lowering=False,
        debug=False,
        enable_asserts=False,
        num_devices=NCORES,
    )
    xqt = nc.dram_tensor("xqt", [F, T], DT, kind="ExternalInput").ap()
    xkvt = nc.dram_tensor("xkvt", [F, TKV], DT, kind="ExternalInput").ap()
    wq = nc.dram_tensor("wq", [F, P], DT, kind="ExternalInput").ap()
    wk = nc.dram_tensor("wk", [F, P], DT, kind="ExternalInput").ap()
    wv = nc.dram_tensor("wv", [F, P], DT, kind="ExternalInput").ap()
    out = nc.dram_tensor("out", [T, P], F32, kind="ExternalOutput").ap()
    sums = nc.dram_tensor("sums", [1, T], F32, kind="ExternalOutput").ap()
    with tile.TileContext(nc) as tc, ExitStack() as ctx:
        _attn_body(ctx, tc, xqt, xkvt, wq, wk, wv, out, sums)
    nc.compile()
    _CACHE[key] = nc
    return nc


def kernel(x, Wq, Wk, Wv, _trace=False):
    _, np_dt = _mm_dtypes()
    nc = _get_compiled()
    wq_c = np.ascontiguousarray(Wq.astype(np_dt))
    wk_c = np.ascontiguousarray(Wk.astype(np_dt))
    wv_c = np.ascontiguousarray(Wv.astype(np_dt))
    xT = [np.ascontiguousarray(x[b].T.astype(np_dt)) for b in range(B)]
    in_maps = []
    for core in range(NCORES):
        b, h = divmod(core, KSPLIT)
        in_maps.append(
            {
                "xqt": xT[b],
                "xkvt": np.ascontiguousarray(xT[b][:, h * TKV : (h + 1) * TKV]),
                "wq": wq_c,
                "wk": wk_c,
                "wv": wv_c,
            }
        )
    res = run_bass_kernel_spmd(
        nc, in_maps, core_ids=list(range(NCORES)), trace=_trace
    )
    out = np.empty((B, T, P), np.float32)
    for b in range(B):
        r0 = res.results[b * KSPLIT]
        r1 = res.results[b * KSPLIT + 1]
        o = r0["out"] + r1["out"]
        s = r0["sums"][0] + r1["sums"][0]
        out[b] = o / s[:, None]
    if _trace:
        return out, res
    return out
